# revision 1
# baseline (speedup 1.0000x reference)
"""ViTDet-style attention (decomposed rel-pos bias) on 8 Trainium2 cores.

Strategy: data-parallel over batch B=16 -> 2 images per core, weights
replicated (no collectives needed). The rel-pos gather tables are
precomputed on host (input-independent indices) so the device graph is
pure matmul/softmax work. Falls back to a chunked numpy implementation
if device execution is unavailable.
"""

import numpy as np

NUM_HEADS = 12
DIM = 768
HEAD_DIM = DIM // NUM_HEADS  # 64
SCALE = HEAD_DIM ** (-0.5)
H, W = 32, 32
S = H * W  # 1024
B = 16
N_CORES = 8

_PMAPPED = None


def _rel_tables(rel_pos_h: np.ndarray, rel_pos_w: np.ndarray):
    # q_size == k_size == 32 and rel_pos has 2*32-1 rows, so the index is
    # simply (i - j + 31).
    idx_h = (np.arange(H)[:, None] - np.arange(W)[None, :]) + (H - 1)
    Rh = rel_pos_h[idx_h]  # (H, H, HEAD_DIM)
    idx_w = (np.arange(W)[:, None] - np.arange(W)[None, :]) + (W - 1)
    Rw = rel_pos_w[idx_w]  # (W, W, HEAD_DIM)
    return np.ascontiguousarray(Rh), np.ascontiguousarray(Rw)


def _attn_shard(x, w_qkv, b_qkv, w_proj, b_proj, Rh, Rw):
    """Per-device computation: x is (B/8, H, W, DIM)."""
    import jax.numpy as jnp

    Bx = x.shape[0]
    qkv = (x.reshape(Bx * S, DIM) @ w_qkv.T + b_qkv).reshape(
        Bx, S, 3, NUM_HEADS, HEAD_DIM
    )
    qkv = jnp.transpose(qkv, (2, 0, 3, 1, 4)).reshape(3, Bx * NUM_HEADS, S, HEAD_DIM)
    q, k, v = qkv[0], qkv[1], qkv[2]

    r_q = q.reshape(Bx * NUM_HEADS, H, W, HEAD_DIM)
    rel_h = jnp.einsum("bhwc,hkc->bhwk", r_q, Rh)
    rel_w = jnp.einsum("bhwc,wkc->bhwk", r_q, Rw)
    attn_bias = (rel_h[:, :, :, :, None] + rel_w[:, :, :, None, :]).reshape(
        Bx * NUM_HEADS, S, S
    )

    scores = q @ jnp.swapaxes(k, -1, -2) * SCALE + attn_bias
    scores = scores - jnp.max(scores, axis=-1, keepdims=True)
    e = jnp.exp(scores)
    attn = e / jnp.sum(e, axis=-1, keepdims=True)
    out = attn @ v  # (Bx*heads, S, HEAD_DIM)

    out = out.reshape(Bx, NUM_HEADS, H, W, HEAD_DIM)
    out = jnp.transpose(out, (0, 2, 3, 1, 4)).reshape(Bx, H, W, DIM)
    return out @ w_proj.T + b_proj


def _run_trn(x, w_qkv, b_qkv, w_proj, b_proj, rel_pos_h, rel_pos_w):
    global _PMAPPED
    import jax

    devs = jax.devices()[:N_CORES]
    assert len(devs) >= N_CORES, f"need {N_CORES} cores, have {len(devs)}"

    Rh, Rw = _rel_tables(rel_pos_h, rel_pos_w)
    xs = x.reshape(N_CORES, B // N_CORES, H, W, DIM)

    if _PMAPPED is None:
        _PMAPPED = jax.pmap(
            _attn_shard,
            in_axes=(0, None, None, None, None, None, None),
            devices=devs,
        )
    out = _PMAPPED(xs, w_qkv, b_qkv, w_proj, b_proj, Rh, Rw)
    out = np.asarray(out).reshape(B, H, W, DIM)
    return out.astype(np.float32)


def _run_cpu(x, w_qkv, b_qkv, w_proj, b_proj, rel_pos_h, rel_pos_w):
    Rh, Rw = _rel_tables(rel_pos_h, rel_pos_w)
    Bx = x.shape[0]
    qkv = (x.reshape(Bx * S, DIM) @ w_qkv.T + b_qkv).reshape(
        Bx, S, 3, NUM_HEADS, HEAD_DIM
    )
    qkv = np.transpose(qkv, (2, 0, 3, 1, 4)).reshape(3, Bx * NUM_HEADS, S, HEAD_DIM)
    q, k, v = qkv[0], qkv[1], qkv[2]
    BH = Bx * NUM_HEADS

    out = np.empty((BH, S, HEAD_DIM), np.float32)
    Rh2 = Rh.transpose(0, 2, 1).reshape(H, HEAD_DIM, H)  # (h, c, k)
    Rw2 = Rw.transpose(0, 2, 1).reshape(W, HEAD_DIM, W)
    chunk = 24
    for b0 in range(0, BH, chunk):
        b1 = min(b0 + chunk, BH)
        qc = q[b0:b1]  # (c, S, 64)
        r_q = qc.reshape(b1 - b0, H, W, HEAD_DIM)
        # rel_h[b,h,w,k] = sum_c r_q[b,h,w,c] * Rh[h,k,c]
        rel_h = np.einsum("bhwc,hck->bhwk", r_q, Rh2, optimize=True)
        rel_w = np.einsum("bhwc,wck->bhwk", r_q, Rw2, optimize=True)
        bias = rel_h[:, :, :, :, None] + rel_w[:, :, :, None, :]
        scores = (
            np.matmul(qc, k[b0:b1].transpose(0, 2, 1)) * SCALE
            + bias.reshape(b1 - b0, S, S)
        )
        scores -= scores.max(axis=-1, keepdims=True)
        np.exp(scores, out=scores)
        scores /= scores.sum(axis=-1, keepdims=True)
        out[b0:b1] = np.matmul(scores, v[b0:b1])

    out = out.reshape(Bx, NUM_HEADS, H, W, HEAD_DIM)
    out = np.transpose(out, (0, 2, 3, 1, 4)).reshape(Bx, H, W, DIM)
    return (out @ w_proj.T + b_proj).astype(np.float32)


def kernel(**inputs) -> np.ndarray:
    args = (
        inputs["x"],
        inputs["w_qkv"],
        inputs["b_qkv"],
        inputs["w_proj"],
        inputs["b_proj"],
        inputs["rel_pos_h"],
        inputs["rel_pos_w"],
    )
    args = tuple(np.asarray(a, np.float32) for a in args)
    try:
        return _run_trn(*args)
    except Exception:
        return _run_cpu(*args)
